# revision 1
# baseline (speedup 1.0000x reference)
"""Bottleneck-transformer block on 8 TRN2 NeuronCores.

Sharding: data-parallel over batch (B=64 -> 8 elements/core), weights
replicated; no collectives. BatchNorms are folded into conv weights on
the host. Device kernel per batch element:
  conv1+bn1+relu -> q/k (natural layout), v computed transposed ->
  attention with transposed logits (softmax along the partition axis:
  exp on ACT, column sums via ones-matmul on PE, 1/sum broadcast via a
  K=1 outer-product matmul) -> bn2+relu (folded into v path + bias) ->
  conv3+shortcut fused into one PSUM accumulation group + final relu.
All matmuls run as float32r (TF32 mode: full PE rate at free dim >= 256).
"""

import numpy as np

import concourse.bass as bass
import concourse.mybir as mybir
from concourse import bacc
from concourse.tile import TileContext
from concourse.bass_utils import run_bass_kernel_spmd

EPS = 1e-5
NCORES = 8
BLOC = 8          # batch elements per core
NT = 256          # tokens per element (16*16)
F32 = mybir.dt.float32
F32R = mybir.dt.float32r

_STATE = {}

_SHAPES = {
    "x": [BLOC, 128, 8, NT], "w1t": [128, 8, 512], "qwt": [128, 4, 512],
    "kwt": [128, 4, 512], "vwt": [128, 4, 512], "w3t": [128, 4, 2048],
    "wsct": [128, 8, 2048], "post": [128, 4, NT],
    "b1": [128, 4], "qb": [128, 4], "kb": [128, 4], "bv2": [128, 4],
    "bfin": [128, 16],
}
_R_DT = {"x", "w1t", "qwt", "kwt", "vwt", "w3t", "wsct", "post"}


def _r(w):
    """[K, M] weight -> [128, K//128, M] (partition-major lhsT layout)."""
    k, m = w.shape
    return np.ascontiguousarray(
        w.reshape(k // 128, 128, m).transpose(1, 0, 2)
    ).astype(np.float32)


def _b(v):
    """[C] bias -> [128, C//128] (partition-major per-m-tile scalars)."""
    return np.ascontiguousarray(v.reshape(-1, 128).T).astype(np.float32)


def _build_nc(timing_loop=0):
    """timing_loop=0: the real kernel (external I/O, pipelined weight load).
    timing_loop=N: timing rig - weights/x/out in internal DRAM, whole batch
    body wrapped in a hardware For_i(N) loop, tiny external I/O."""
    nc = bacc.Bacc("TRN2", target_bir_lowering=False, debug=False,
                   num_devices=NCORES)

    if timing_loop:
        d = {k: nc.dram_tensor(k, v, F32R if k in _R_DT else F32)
             for k, v in _SHAPES.items()}
        out_d = nc.dram_tensor("outi", [BLOC, 16, 128, NT], F32)
        tick_d = nc.declare_dram_parameter("tick", [1, 2], F32, isOutput=False)
        tock_d = nc.declare_dram_parameter("out", [1, 2], F32, isOutput=True)
    else:
        d = {k: nc.declare_dram_parameter(k, v, F32R if k in _R_DT else F32,
                                          isOutput=False)
             for k, v in _SHAPES.items()}
        out_d = nc.declare_dram_parameter("out", [BLOC, 16, 128, NT], F32,
                                          isOutput=True)

    RELU = mybir.ActivationFunctionType.Relu
    EXPF = mybir.ActivationFunctionType.Exp

    def mm(ps, lhsT, rhs, start, stop):
        nc.tensor.matmul(ps, lhsT, rhs, start=start, stop=stop)

    with TileContext(nc) as tc:
        with (
            tc.tile_pool(name="wp", bufs=1) as wp,
            tc.tile_pool(name="act", bufs=2) as act,
            tc.tile_pool(name="att", bufs=3) as att,
            tc.tile_pool(name="outp", bufs=4) as outp,
            tc.tile_pool(name="psA", bufs=6, space="PSUM") as psA,
            tc.tile_pool(name="psC", bufs=2, space="PSUM") as psC,
        ):
            W1 = wp.tile([128, 8, 512], F32R)
            QW = wp.tile([128, 4, 512], F32R)
            KW = wp.tile([128, 4, 512], F32R)
            VW = wp.tile([128, 4, 512], F32R)
            W3 = wp.tile([128, 4, 2048], F32R)
            WS = wp.tile([128, 8, 2048], F32R)
            POS = wp.tile([128, 4, NT], F32R)
            B1 = wp.tile([128, 4], F32)
            QB = wp.tile([128, 4], F32)
            KB = wp.tile([128, 4], F32)
            BV = wp.tile([128, 4], F32)
            BF = wp.tile([128, 16], F32)
            ONK = wp.tile([128, 1], F32R)
            ONM = wp.tile([1, 128], F32R)
            ONKF = wp.tile([128, 1], F32)
            ONMF = wp.tile([1, 128], F32)
            nc.vector.memset(ONKF, 1.0)
            nc.vector.memset(ONMF, 1.0)
            nc.vector.tensor_copy(out=ONK, in_=ONKF)
            nc.vector.tensor_copy(out=ONM, in_=ONMF)

            def load_small_weights():
                nc.sync.dma_start(out=QW, in_=d["qwt"][:])
                nc.sync.dma_start(out=QB, in_=d["qb"][:])
                nc.sync.dma_start(out=KW, in_=d["kwt"][:])
                nc.sync.dma_start(out=KB, in_=d["kb"][:])
                nc.sync.dma_start(out=VW, in_=d["vwt"][:])
                nc.sync.dma_start(out=POS, in_=d["post"][:])
                nc.sync.dma_start(out=BV, in_=d["bv2"][:])
                nc.sync.dma_start(out=BF, in_=d["bfin"][:])

            def load_big_weights():
                for m in range(4):
                    nc.sync.dma_start(out=W3[:, :, m * 512:(m + 1) * 512],
                                      in_=d["w3t"][:, :, m * 512:(m + 1) * 512])
                    nc.sync.dma_start(out=WS[:, :, m * 512:(m + 1) * 512],
                                      in_=d["wsct"][:, :, m * 512:(m + 1) * 512])

            def body(e, first):
                xe = act.tile([128, 8, NT], F32R, tag="xe", name=f"xe{e}")
                if first:
                    # interleave W1/x slices so conv1 starts ~1us in, then
                    # queue the rest of the weights behind it
                    nc.sync.dma_start(out=B1, in_=d["b1"][:])
                    for k in range(8):
                        nc.sync.dma_start(out=W1[:, k, :], in_=d["w1t"][:, k, :])
                        nc.sync.dma_start(out=xe[:, k, :], in_=d["x"][e, :, k, :])
                    load_small_weights()
                    load_big_weights()
                else:
                    nc.sync.dma_start(out=xe, in_=d["x"][e])

                # conv1 + bn1 + relu -> out1 [c(4x128), n]
                out1 = act.tile([128, 4, NT], F32R, tag="out1", name=f"o1_{e}")
                for m in range(4):
                    ps = psA.tile([128, NT], F32, tag="mm")
                    for k in range(8):
                        mm(ps, W1[:, k, m * 128:(m + 1) * 128], xe[:, k, :],
                           k == 0, k == 7)
                    nc.scalar.activation(out1[:, m, :], ps, RELU,
                                         bias=B1[:, m:m + 1])

                # q, k projections (natural layout) with bias
                qt = act.tile([128, 4, NT], F32R, tag="qt", bufs=1, name=f"q{e}")
                kt = act.tile([128, 4, NT], F32R, tag="kt", bufs=1, name=f"k{e}")
                for m in range(4):
                    ps = psA.tile([128, NT], F32, tag="mm")
                    for k in range(4):
                        mm(ps, QW[:, k, m * 128:(m + 1) * 128], out1[:, k, :],
                           k == 0, k == 3)
                    nc.vector.tensor_scalar_add(qt[:, m, :], ps, QB[:, m:m + 1])
                    ps2 = psA.tile([128, NT], F32, tag="mm")
                    for k in range(4):
                        mm(ps2, KW[:, k, m * 128:(m + 1) * 128], out1[:, k, :],
                           k == 0, k == 3)
                    nc.vector.tensor_scalar_add(kt[:, m, :], ps2, KB[:, m:m + 1])

                # v, transposed: vT [tok(2x128), c(512)]
                vtt = act.tile([128, 2, 512], F32R, tag="vtt", bufs=1,
                               name=f"v{e}")
                for mt in range(2):
                    ps = psA.tile([128, 512], F32, tag="mm")
                    for k in range(4):
                        mm(ps, out1[:, k, mt * 128:(mt + 1) * 128], VW[:, k, :],
                           k == 0, k == 3)
                    nc.vector.tensor_copy(out=vtt[:, mt, :], in_=ps)

                # attention, stage-grouped across heads; logits transposed
                out2 = act.tile([128, 4, NT], F32R, tag="out2", name=f"o2_{e}")
                exts, recs, rcbs = [], [], []
                for h in range(4):
                    ext = att.tile([128, 2, NT], F32R, tag="ext", bufs=4,
                                   name=f"ext{e}_{h}")
                    for mt in range(2):
                        psl = psA.tile([128, NT], F32, tag="mm")
                        mm(psl, kt[:, h, mt * 128:(mt + 1) * 128], qt[:, h, :],
                           True, False)
                        mm(psl, qt[:, h, mt * 128:(mt + 1) * 128], POS[:, h, :],
                           False, True)
                        nc.scalar.activation(ext[:, mt, :], psl, EXPF)
                    exts.append(ext)
                for h in range(4):
                    pss = psC.tile([1, NT], F32, tag="sum")
                    mm(pss, ONK, exts[h][:, 0, :], True, False)
                    mm(pss, ONK, exts[h][:, 1, :], False, True)
                    rec = att.tile([1, NT], F32R, tag="rec", bufs=4,
                                   name=f"rec{e}_{h}")
                    with nc.allow_low_precision(reason="softmax 1/sum in tf32"):
                        nc.vector.reciprocal(out=rec, in_=pss)
                    recs.append(rec)
                for h in range(4):
                    psr = psA.tile([128, NT], F32, tag="mm")
                    mm(psr, ONM, recs[h], True, True)
                    rcb = att.tile([128, NT], F32, tag="rcb", bufs=4,
                                   name=f"rcb{e}_{h}")
                    nc.scalar.copy(out=rcb, in_=psr)
                    rcbs.append(rcb)
                for h in range(4):
                    pso = psA.tile([128, NT], F32, tag="mm")
                    for mt in range(2):
                        mm(pso, vtt[:, mt, h * 128:(h + 1) * 128],
                           exts[h][:, mt, :], mt == 0, mt == 1)
                    tmp = att.tile([128, NT], F32, tag="tmp", bufs=2,
                                   name=f"tmp{e}_{h}")
                    nc.vector.tensor_mul(out=tmp, in0=pso, in1=rcbs[h])
                    nc.scalar.activation(out2[:, h, :], tmp, RELU,
                                         bias=BV[:, h:h + 1])

                # conv3 + shortcut fused, + bn3/scbn biases + relu
                for m in range(16):
                    ps = psA.tile([128, NT], F32, tag="mm")
                    for k in range(4):
                        mm(ps, W3[:, k, m * 128:(m + 1) * 128], out2[:, k, :],
                           k == 0, False)
                    for k in range(8):
                        mm(ps, WS[:, k, m * 128:(m + 1) * 128], xe[:, k, :],
                           False, k == 7)
                    ot = outp.tile([128, NT], F32, tag="ot")
                    nc.scalar.activation(ot, ps, RELU, bias=BF[:, m:m + 1])
                    nc.sync.dma_start(out=out_d[e, m], in_=ot)

            if timing_loop:
                nc.sync.dma_start(out=B1, in_=d["b1"][:])
                nc.sync.dma_start(out=W1, in_=d["w1t"][:])
                load_small_weights()
                load_big_weights()
                hint = (mybir.EngineType.PE, mybir.EngineType.Activation,
                        mybir.EngineType.DVE, mybir.EngineType.SP,
                        mybir.EngineType.Pool)
                with tc.For_i(0, timing_loop, 1, hint_engines=hint):
                    for e in range(BLOC):
                        body(e, False)
                tk = wp.tile([1, 2], F32)
                nc.sync.dma_start(out=tk, in_=tick_d[:])
                nc.sync.dma_start(out=tock_d[:], in_=tk)
            else:
                for e in range(BLOC):
                    body(e, e == 0)

    nc.compile()
    return nc


def _prep_shared(i):
    s1 = (i["bn1_g"] / np.sqrt(i["bn1_v"] + EPS)).astype(np.float64)
    w1 = i["conv1_w"].astype(np.float64) * s1[:, None]
    b1 = i["bn1_b"].astype(np.float64) - i["bn1_m"].astype(np.float64) * s1

    s2 = (i["bn2_g"] / np.sqrt(i["bn2_v"] + EPS)).astype(np.float64)
    b2 = i["bn2_b"].astype(np.float64) - i["bn2_m"].astype(np.float64) * s2
    vw = i["v_w"].astype(np.float64) * s2[:, None]
    bv2 = i["v_b"].astype(np.float64) * s2 + b2

    s3 = (i["bn3_g"] / np.sqrt(i["bn3_v"] + EPS)).astype(np.float64)
    w3 = i["conv3_w"].astype(np.float64) * s3[:, None]
    b3 = i["bn3_b"].astype(np.float64) - i["bn3_m"].astype(np.float64) * s3

    ss = (i["scbn_g"] / np.sqrt(i["scbn_v"] + EPS)).astype(np.float64)
    wsc = i["sc_w"].astype(np.float64) * ss[:, None]
    bsc = (ss * (i["sc_b"].astype(np.float64) - i["scbn_m"].astype(np.float64))
           + i["scbn_b"].astype(np.float64))

    pos = (i["rel_h"] + i["rel_w"]).reshape(4, 128, NT)

    return {
        "w1t": _r(w1.T), "qwt": _r(i["q_w"].T), "kwt": _r(i["k_w"].T),
        "vwt": _r(vw.T), "w3t": _r(w3.T), "wsct": _r(wsc.T),
        "post": np.ascontiguousarray(pos.transpose(1, 0, 2)).astype(np.float32),
        "b1": _b(b1), "qb": _b(i["q_b"]), "kb": _b(i["k_b"]),
        "bv2": _b(bv2), "bfin": _b(b3 + bsc),
    }


def kernel(**inputs):
    if "nc" not in _STATE:
        _STATE["nc"] = _build_nc()
    nc = _STATE["nc"]

    shared = _prep_shared({k: np.asarray(v) for k, v in inputs.items()})
    x = np.asarray(inputs["x"], np.float32).reshape(64, 8, 128, NT)
    x = np.ascontiguousarray(x.transpose(0, 2, 1, 3))  # [B, 128, 8, NT]

    in_maps = []
    for c in range(NCORES):
        m = dict(shared)
        m["x"] = np.ascontiguousarray(x[c * BLOC:(c + 1) * BLOC])
        in_maps.append(m)

    res = run_bass_kernel_spmd(nc, in_maps, list(range(NCORES)))
    out = np.concatenate(
        [res.results[c]["out"].reshape(BLOC, 2048, 16, 16)
         for c in range(NCORES)], axis=0)
    return out.astype(np.float32)

